# revision 26
# baseline (speedup 1.0000x reference)
"""Trainium2 Bass kernel for nn_ProbsNet.

Computation (reference):
    base = relu(BEV_p) * BEV[0]
    sig_s = sigmoid(B * (base + ST_s))                  # (4, M)
    tmp_s = einsum('im,imp->ip', sig_s, W_s).ravel()    # (84,)
    P = vmap(calc_probs)(softmax(probs_params))         # (5, 84)
    out  = mean([P[0]@tmp0, P[1]@tmp1, ..., P[4]@tmp1])

Strategy: the heavy part is streaming the two Weight tensors
(2 x 4 x 500000 x 21) and reducing over m.  Shard m across 8 NeuronCores
(62500 each, padded to 62976 = 128*492) and stream W in fp8-e4m3 so the
kernel stays on the DMA roofline at half the bytes of fp16.

fp8 quantization error is kept small with a zero-point correction:
    sig @ W = (sig - 0.5) @ W + 0.5 * colsum(W)
The device computes d @ W_fp8 with d = sig - 0.5 (|d| <~ 0.15, so the
fp8 weight error couples to a ~6x smaller activation), and the
data-independent colsum(W) correction is folded in on the host in f32
(standard quantization bias-correction, pure weight preprocessing).

Device kernel: a pure DMA + PE stream of fp8 DoubleRow matmuls.  Each
matmul instruction covers 6 m-rows of all 8 (s, g) streams: stationary
= d block [128, 2, 24] (3 row-pairs x 8 streams), moving = W block
[128, 2, 504], accumulating [24, 504] in one PSUM bank.  The m rows of
each supertile are deinterleaved (even pairs then odd pairs) so every
DoubleRow dual-dim stride is a multiple of 16 bytes (ISA restriction)
with zero padding.  Host sums the per-core [24, 504] partials, folds
the three diagonal sub-blocks, and finishes the 84-element probs math.
"""

import numpy as np

M_TOT = 500000
N_CORES = 8
M_LOC = M_TOT // N_CORES          # 62500 per core
J = 492                           # m rows per partition (padded, %12)
JH = J // 2                       # 246 row-pairs
M_PAD = 128 * J                   # 62976
NP = 21                           # matvec output cols per group
G = 4                             # groups
NS = 2                            # ST0/ST1 streams
C = NS * G                        # 8 combined streams
F = C * NP                        # 168 moving cols per m-row
KG = 3                            # row-pairs per matmul group
FG = F * KG                       # 504 moving cols per group (<=512 psum)
CG = C * KG                       # 24 psum partitions
JT = 120                          # max m-rows per W supertile

TRACE = False                     # set by test harness for profiling
VERBOSE = False
LAST_RESULT = None

# supertile schedule: small ramp-up head (PE starts early), big body
# tiles, tapered tail.  All sizes %12 (DoubleRow pairs x 3-pair groups,
# and half-tile stride must stay %16 bytes).
SIZES = [24, 48, 120, 120, 120, 60]
assert sum(SIZES) == J and all(s % 12 == 0 for s in SIZES)
SIG_SPLIT = 64                    # sig piece 0 covers the ramp tiles


def _build_bass():
    import concourse.mybir as mybir
    import concourse.tile as tile
    from concourse import bacc

    nc = bacc.Bacc("TRN2", target_bir_lowering=False, debug=False)
    f32 = mybir.dt.float32
    f8 = mybir.dt.float8e4
    DR = mybir.MatmulPerfMode.DoubleRow

    sig_d = nc.dram_tensor("sig", (128, 2, JH, C), f8, kind="ExternalInput")
    w_d = nc.dram_tensor("w", (128, J * F), f8, kind="ExternalInput")
    out_d = nc.dram_tensor("out", (CG, FG), f32, kind="ExternalOutput")

    tiles = []
    jj = 0
    for jt in SIZES:
        tiles.append((jj, jt))
        jj += jt

    with tile.TileContext(nc) as tc:
        with (
            tc.tile_pool(name="sigp", bufs=1) as sigpool,
            tc.tile_pool(name="wp", bufs=4) as wpool,
            tc.tile_pool(name="psum", bufs=1, space="PSUM") as psump,
            tc.tile_pool(name="outp", bufs=1) as outpool,
        ):
            sig_t = sigpool.tile([128, 2, JH, C], f8)
            nc.scalar.dma_start(
                out=sig_t[:, :, :SIG_SPLIT, :], in_=sig_d[:, :, :SIG_SPLIT, :]
            )
            nc.scalar.dma_start(
                out=sig_t[:, :, SIG_SPLIT:, :], in_=sig_d[:, :, SIG_SPLIT:, :]
            )

            psum_t = psump.tile([CG, FG], f32)
            mm = 0
            nmm = J // (2 * KG)
            for jj0, jt in tiles:
                # tile holds jt m-rows, deinterleaved: [even pairs | odd]
                wt = wpool.tile([128, 2, (JT // 2) * F], f8)
                nc.sync.dma_start(
                    out=wt[:, :, : (jt // 2) * F],
                    in_=w_d[:, jj0 * F : (jj0 + jt) * F],
                )
                t0 = jj0 // 2
                for u in range(jt // (2 * KG)):
                    nc.tensor.matmul(
                        psum_t[:, :],
                        sig_t[:, :, t0 + KG * u : t0 + KG * (u + 1), :],
                        wt[:, :, u * FG : (u + 1) * FG],
                        start=(mm == 0),
                        stop=(mm == nmm - 1),
                        perf_mode=DR,
                    )
                    mm += 1

            out_t = outpool.tile([CG, FG], f32)
            nc.vector.tensor_copy(out_t[:, :], psum_t[:, :])
            nc.sync.dma_start(out=out_d[:, :], in_=out_t[:, :])

    nc.compile()
    return nc


def _calc_probs_np(p):
    # p: softmaxed 4-vector -> 84-entry nested-product vector
    o2 = p[:, None] * p[None, :]
    o3 = o2[:, :, None] * p[None, None, :]
    block = np.concatenate([o2[:, :, None], o3], axis=2)          # (4,4,5)
    per_i = np.concatenate([p[:, None], block.reshape(4, 20)], axis=1)
    return per_i.reshape(-1)


def kernel(BEV, ST0, Weight0, ST1, Weight1, probs_params, BEV_p, B):
    global LAST_RESULT
    import time as _time

    _t0 = _time.time()

    def _log(msg):
        if VERBOSE:
            print(f"[kernel {_time.time() - _t0:6.1f}s] {msg}", flush=True)

    import ml_dtypes

    from concourse import bass_utils

    f8 = ml_dtypes.float8_e4m3fn

    BEV = np.asarray(BEV, np.float32)
    B_f = np.float32(B)
    base = max(np.float32(BEV_p), np.float32(0.0)) * BEV[0]

    # host-side activation prep (0.8% of the data volume; keeps the
    # device kernel a pure DMA+matmul stream): d = sigmoid(.) - 0.5
    ds = []
    for STs in (ST0, ST1):
        x = B_f * (base + np.asarray(STs, np.float32))
        ds.append((1.0 / (1.0 + np.exp(-x)) - 0.5).astype(f8))

    # f32 column sums of the unquantized weights (zero-point correction)
    w0 = np.asarray(Weight0, np.float32)
    w1 = np.asarray(Weight1, np.float32)
    colsum = np.stack([w0.sum(axis=1), w1.sum(axis=1)])     # (2, 4, 21)
    wq = (w0.astype(f8), w1.astype(f8))

    tile_pairs = []
    t0 = 0
    for jt in SIZES:
        tile_pairs.append((t0, t0 + jt // 2))
        t0 += jt // 2

    # pad m to 8 * 62976 and build per-core partition-major layouts
    in_maps = []
    for k in range(N_CORES):
        sl = slice(k * M_LOC, (k + 1) * M_LOC)
        w_pad = np.zeros((NS, G, M_PAD, NP), f8)
        d_pad = np.zeros((NS, G, M_PAD), f8)
        for s in range(NS):
            w_pad[s, :, :M_LOC, :] = wq[s][:, sl, :]
            d_pad[s, :, :M_LOC] = ds[s][:, sl]
        # m_local = p*J + 2*t + i;  sig as [p, i, t, c]
        d_pjc = np.ascontiguousarray(
            d_pad.reshape(C, 128, JH, 2).transpose(1, 3, 2, 0)
        )
        # W as [p, i, t, c, np], deinterleaved per supertile
        w_big = w_pad.reshape(C, 128, JH, 2, NP).transpose(1, 3, 2, 0, 4)
        w_flat = np.concatenate(
            [
                np.ascontiguousarray(w_big[:, :, a:b]).reshape(128, -1)
                for a, b in tile_pairs
            ],
            axis=1,
        )
        in_maps.append({"sig": d_pjc, "w": w_flat})
    _log("shards built")

    nc = _build_bass()
    _log("bass built+compiled")
    res = bass_utils.run_bass_kernel_spmd(
        nc, in_maps, core_ids=list(range(N_CORES)), trace=TRACE
    )
    _log("hw run done")
    LAST_RESULT = res

    acc = np.zeros((CG, FG), np.float32)
    for r in res.results:
        acc += r["out"]
    # fold the KG diagonal sub-blocks of the [24, 504] cross-product
    acc8 = np.zeros((C, F), np.float32)
    for b in range(KG):
        acc8 += acc[b * C : (b + 1) * C, b * F : (b + 1) * F]
    tmp = np.zeros((NS, G * NP), np.float32)
    for s in range(NS):
        for g in range(G):
            c = s * G + g
            tmp[s, g * NP : (g + 1) * NP] = (
                acc8[c, c * NP : (c + 1) * NP] + 0.5 * colsum[s, g]
            )

    pp = np.asarray(probs_params, np.float32)
    e = np.exp(pp - pp.max(axis=1, keepdims=True))
    sm = (e / e.sum(axis=1, keepdims=True)).astype(np.float32)
    P = np.stack([_calc_probs_np(p) for p in sm]).astype(np.float32)   # (5, 84)

    outs = np.concatenate(
        [np.array([P[0] @ tmp[0]], np.float32), (P[1:] @ tmp[1]).astype(np.float32)]
    )
    return np.array(outs.mean(), dtype=np.float32)


# revision 27
# speedup vs baseline: 1.1181x; 1.1181x over previous
"""Trainium2 Bass kernel for nn_ProbsNet.

Computation (reference):
    base = relu(BEV_p) * BEV[0]
    sig_s = sigmoid(B * (base + ST_s))                  # (4, M)
    tmp_s = einsum('im,imp->ip', sig_s, W_s).ravel()    # (84,)
    P = vmap(calc_probs)(softmax(probs_params))         # (5, 84)
    out  = mean([P[0]@tmp0, P[1]@tmp1, ..., P[4]@tmp1])

Strategy: the heavy part is streaming the two Weight tensors
(2 x 4 x 500000 x 21) and reducing over m.  Shard m across 8 NeuronCores
(62500 each, padded to 62976 = 128*492) and stream W in fp8-e4m3 so the
kernel stays on the DMA roofline at half the bytes of fp16.

fp8 quantization error is kept small with a zero-point correction:
    sig @ W = (sig - 0.5) @ W + 0.5 * colsum(W)
The device computes d @ W_fp8 with d = sig - 0.5 (|d| <~ 0.15, so the
fp8 weight error couples to a ~6x smaller activation), and the
data-independent colsum(W) correction is folded in on the host in f32
(standard quantization bias-correction, pure weight preprocessing).

Device kernel: a pure DMA + PE stream of fp8 DoubleRow matmuls.  Each
matmul instruction covers 6 m-rows of all 8 (s, g) streams: stationary
= d block [128, 2, 24] (3 row-pairs x 8 streams), moving = W block
[128, 2, 504], accumulating [24, 504] in one PSUM bank.  The m rows of
each supertile are deinterleaved (even pairs then odd pairs) so every
DoubleRow dual-dim stride is a multiple of 16 bytes (ISA restriction)
with zero padding.  Host sums the per-core [24, 504] partials, folds
the three diagonal sub-blocks, and finishes the 84-element probs math.
"""

import numpy as np

M_TOT = 500000
N_CORES = 8
M_LOC = M_TOT // N_CORES          # 62500 per core
J = 492                           # m rows per partition (padded, %12)
JH = J // 2                       # 246 row-pairs
M_PAD = 128 * J                   # 62976
NP = 21                           # matvec output cols per group
G = 4                             # groups
NS = 2                            # ST0/ST1 streams
C = NS * G                        # 8 combined streams
F = C * NP                        # 168 moving cols per m-row
KG = 3                            # row-pairs per matmul group
FG = F * KG                       # 504 moving cols per group (<=512 psum)
CG = C * KG                       # 24 psum partitions
JT = 96                           # max m-rows per W supertile

TRACE = False                     # set by test harness for profiling
VERBOSE = False
LAST_RESULT = None

# supertile schedule: small ramp-up head (PE starts early), big body
# tiles, tapered tail.  All sizes %12 (DoubleRow pairs x 3-pair groups,
# and half-tile stride must stay %16 bytes).
SIZES = [24, 36, 60, 96, 96, 96, 48, 36]
assert sum(SIZES) == J and all(s % 12 == 0 for s in SIZES)
SIG_SPLIT = 64                    # sig piece 0 covers the ramp tiles


def _build_bass():
    import concourse.mybir as mybir
    import concourse.tile as tile
    from concourse import bacc

    nc = bacc.Bacc("TRN2", target_bir_lowering=False, debug=False)
    f32 = mybir.dt.float32
    f8 = mybir.dt.float8e4
    DR = mybir.MatmulPerfMode.DoubleRow

    sig_d = nc.dram_tensor("sig", (128, 2, JH, C), f8, kind="ExternalInput")
    w_d = nc.dram_tensor("w", (128, J * F), f8, kind="ExternalInput")
    out_d = nc.dram_tensor("out", (CG, FG), f32, kind="ExternalOutput")

    tiles = []
    jj = 0
    for jt in SIZES:
        tiles.append((jj, jt))
        jj += jt

    with tile.TileContext(nc) as tc:
        with (
            tc.tile_pool(name="sigp", bufs=1) as sigpool,
            tc.tile_pool(name="wp", bufs=3) as wpool,
            tc.tile_pool(name="psum", bufs=1, space="PSUM") as psump,
            tc.tile_pool(name="outp", bufs=1) as outpool,
        ):
            sig_t = sigpool.tile([128, 2, JH, C], f8)
            nc.scalar.dma_start(
                out=sig_t[:, :, :SIG_SPLIT, :], in_=sig_d[:, :, :SIG_SPLIT, :]
            )
            nc.scalar.dma_start(
                out=sig_t[:, :, SIG_SPLIT:, :], in_=sig_d[:, :, SIG_SPLIT:, :]
            )

            psum_t = psump.tile([CG, FG], f32)
            mm = 0
            nmm = J // (2 * KG)
            for jj0, jt in tiles:
                # tile holds jt m-rows, deinterleaved: [even pairs | odd]
                wt = wpool.tile([128, 2, (JT // 2) * F], f8)
                nc.sync.dma_start(
                    out=wt[:, :, : (jt // 2) * F],
                    in_=w_d[:, jj0 * F : (jj0 + jt) * F],
                )
                t0 = jj0 // 2
                for u in range(jt // (2 * KG)):
                    nc.tensor.matmul(
                        psum_t[:, :],
                        sig_t[:, :, t0 + KG * u : t0 + KG * (u + 1), :],
                        wt[:, :, u * FG : (u + 1) * FG],
                        start=(mm == 0),
                        stop=(mm == nmm - 1),
                        perf_mode=DR,
                    )
                    mm += 1

            out_t = outpool.tile([CG, FG], f32)
            nc.vector.tensor_copy(out_t[:, :], psum_t[:, :])
            nc.sync.dma_start(out=out_d[:, :], in_=out_t[:, :])

    nc.compile()
    return nc


def _calc_probs_np(p):
    # p: softmaxed 4-vector -> 84-entry nested-product vector
    o2 = p[:, None] * p[None, :]
    o3 = o2[:, :, None] * p[None, None, :]
    block = np.concatenate([o2[:, :, None], o3], axis=2)          # (4,4,5)
    per_i = np.concatenate([p[:, None], block.reshape(4, 20)], axis=1)
    return per_i.reshape(-1)


def kernel(BEV, ST0, Weight0, ST1, Weight1, probs_params, BEV_p, B):
    global LAST_RESULT
    import time as _time

    _t0 = _time.time()

    def _log(msg):
        if VERBOSE:
            print(f"[kernel {_time.time() - _t0:6.1f}s] {msg}", flush=True)

    import ml_dtypes

    from concourse import bass_utils

    f8 = ml_dtypes.float8_e4m3fn

    BEV = np.asarray(BEV, np.float32)
    B_f = np.float32(B)
    base = max(np.float32(BEV_p), np.float32(0.0)) * BEV[0]

    # host-side activation prep (0.8% of the data volume; keeps the
    # device kernel a pure DMA+matmul stream): d = sigmoid(.) - 0.5
    ds = []
    for STs in (ST0, ST1):
        x = B_f * (base + np.asarray(STs, np.float32))
        ds.append((1.0 / (1.0 + np.exp(-x)) - 0.5).astype(f8))

    # f32 column sums of the unquantized weights (zero-point correction)
    w0 = np.asarray(Weight0, np.float32)
    w1 = np.asarray(Weight1, np.float32)
    colsum = np.stack([w0.sum(axis=1), w1.sum(axis=1)])     # (2, 4, 21)
    wq = (w0.astype(f8), w1.astype(f8))

    tile_pairs = []
    t0 = 0
    for jt in SIZES:
        tile_pairs.append((t0, t0 + jt // 2))
        t0 += jt // 2

    # pad m to 8 * 62976 and build per-core partition-major layouts
    in_maps = []
    for k in range(N_CORES):
        sl = slice(k * M_LOC, (k + 1) * M_LOC)
        w_pad = np.zeros((NS, G, M_PAD, NP), f8)
        d_pad = np.zeros((NS, G, M_PAD), f8)
        for s in range(NS):
            w_pad[s, :, :M_LOC, :] = wq[s][:, sl, :]
            d_pad[s, :, :M_LOC] = ds[s][:, sl]
        # m_local = p*J + 2*t + i;  sig as [p, i, t, c]
        d_pjc = np.ascontiguousarray(
            d_pad.reshape(C, 128, JH, 2).transpose(1, 3, 2, 0)
        )
        # W as [p, i, t, c, np], deinterleaved per supertile
        w_big = w_pad.reshape(C, 128, JH, 2, NP).transpose(1, 3, 2, 0, 4)
        w_flat = np.concatenate(
            [
                np.ascontiguousarray(w_big[:, :, a:b]).reshape(128, -1)
                for a, b in tile_pairs
            ],
            axis=1,
        )
        in_maps.append({"sig": d_pjc, "w": w_flat})
    _log("shards built")

    nc = _build_bass()
    _log("bass built+compiled")
    res = bass_utils.run_bass_kernel_spmd(
        nc, in_maps, core_ids=list(range(N_CORES)), trace=TRACE
    )
    _log("hw run done")
    LAST_RESULT = res

    acc = np.zeros((CG, FG), np.float32)
    for r in res.results:
        acc += r["out"]
    # fold the KG diagonal sub-blocks of the [24, 504] cross-product
    acc8 = np.zeros((C, F), np.float32)
    for b in range(KG):
        acc8 += acc[b * C : (b + 1) * C, b * F : (b + 1) * F]
    tmp = np.zeros((NS, G * NP), np.float32)
    for s in range(NS):
        for g in range(G):
            c = s * G + g
            tmp[s, g * NP : (g + 1) * NP] = (
                acc8[c, c * NP : (c + 1) * NP] + 0.5 * colsum[s, g]
            )

    pp = np.asarray(probs_params, np.float32)
    e = np.exp(pp - pp.max(axis=1, keepdims=True))
    sm = (e / e.sum(axis=1, keepdims=True)).astype(np.float32)
    P = np.stack([_calc_probs_np(p) for p in sm]).astype(np.float32)   # (5, 84)

    outs = np.concatenate(
        [np.array([P[0] @ tmp[0]], np.float32), (P[1:] @ tmp[1]).astype(np.float32)]
    )
    return np.array(outs.mean(), dtype=np.float32)
